# revision 16
# baseline (speedup 1.0000x reference)
"""GQA attention block (B=2,S=2048,D=4096,H=32,KV=8,HD=128) on 8 TRN2 NeuronCores.

Sharding: 8-way tensor parallel over heads. Core c owns kv-head c and q-heads
4c..4c+3 (wq/wk/wv column-sharded, wo row-sharded). The full-width Q/K
layernorms need cross-core mean/var, done with one tiny (64KB) on-device
AllReduce of per-token partial sums. Each core emits a partial [T,D] output
(its wo row-slice contribution); the host sums the 8 partials.

Device pipeline per core (all matmuls bf16/fp16, f32 accumulation):
  1. Q projection (N=512) interleaved with a fused [wk|wv] projection (N=256)
     so every x-chunk stationary is loaded once and reused by both streams.
     x arrives in 256-token pair loads (512B runs, full DMA rate) on the sync
     queue; weights stream on the gpsimd queue. LN partial stats per tile;
     two AllReduce halves overlap compute.
  2. LN apply + RoPE (shared cos/sin across q heads); transposes to [hd,t]
     via the DMA XBAR (scalar queue) instead of the PE array.
  3. Attention per (b,h): scores^T = k_tile^T q, exp on ACT into fp16,
     attn@V with fp16 v stationary. Softmax sums via an fp16 pairwise adder
     tree on DVE (2x rate) plus one all-ones matmul that both combines and
     broadcasts the denominator; fast approximate reciprocal normalizes in
     the psum->sbuf copyback.
  4. Output projection tiles woven into the attention fill slots (keeps PE
     busy while ACT runs exp, removes the tail); psum->sbuf casts on ACT.
"""

from contextlib import ExitStack

import numpy as np
import ml_dtypes

import concourse.bass as bass
import concourse.mybir as mybir
import concourse.tile as tile
from concourse import bacc
from concourse import bass_utils
from concourse.bass import ts, ds
from concourse.masks import make_identity

BF16 = mybir.dt.bfloat16
F16 = mybir.dt.float16
F32 = mybir.dt.float32
AF = mybir.ActivationFunctionType
ALU = mybir.AluOpType
AX = mybir.AxisListType

B, S, D = 2, 2048, 4096
T = B * S                 # 4096 tokens
H, KV, HD = 32, 8, 128
NCORES = 8
HQ = H // NCORES          # 4 q heads per core
EQ = HQ * HD              # 512
NT = T // 128             # 32 token tiles
ND = D // 128             # 32 contraction chunks
ST = S // 128             # 16 seq tiles per batch
NQB = S // 512            # 4 q-blocks per (b,h)
EPS = 1e-5
SHIFT = 4.0               # softmax shift: exp(s-4) stays in fp16 normal range

PROFILE = False
LAST_EXEC_NS = None
LAST_TRACE_DIR = None
_CACHE = {}


def flat2(ap):  # flatten all free dims -> [P, prod(free)]
    n = len(ap.shape)
    if n == 2:
        return ap
    names = " ".join(f"d{i}" for i in range(n - 1))
    return ap.rearrange(f"p {names} -> p ({names})")


class _Ctx:
    pass


def _build():
    if "nc" in _CACHE:
        return _CACHE["nc"]
    nc = bacc.Bacc("TRN2", target_bir_lowering=False, debug=False,
                   num_devices=NCORES)

    g = _Ctx()
    g.xT_d = nc.dram_tensor("xT", [128, ND, T], BF16, kind="ExternalInput")
    g.wqT_d = nc.dram_tensor("wqT", [128, ND, EQ], BF16, kind="ExternalInput")
    g.wkvT_d = nc.dram_tensor("wkvT", [128, ND, 2 * HD], BF16,
                              kind="ExternalInput")
    g.woT_d = nc.dram_tensor("woT", [128, HQ, D], BF16, kind="ExternalInput")
    g.cosk_d = nc.dram_tensor("cosk", [T, 64, 2], BF16, kind="ExternalInput")
    g.sink_d = nc.dram_tensor("sink", [T, 64, 2], BF16, kind="ExternalInput")
    g.qw_d = nc.dram_tensor("qw", [1, EQ], F32, kind="ExternalInput")
    g.qb_d = nc.dram_tensor("qb", [1, EQ], F32, kind="ExternalInput")
    g.kw_d = nc.dram_tensor("kw", [1, HD], F32, kind="ExternalInput")
    g.kb_d = nc.dram_tensor("kb", [1, HD], F32, kind="ExternalInput")
    g.out_d = nc.dram_tensor("out", [T, D], BF16, kind="ExternalOutput")

    with tile.TileContext(nc) as tc:
        _emit(nc, tc, g)
    nc.compile()
    _CACHE["nc"] = nc
    return nc


def _emit(nc, tc, g):
    ctx = ExitStack()
    with ctx:
        cpool = ctx.enter_context(tc.tile_pool(name="cpool", bufs=1))
        persist = ctx.enter_context(tc.tile_pool(name="persist", bufs=1))
        ardram = ctx.enter_context(
            tc.tile_pool(name="ardram", bufs=1, space="DRAM"))
        p2 = ctx.enter_context(tc.tile_pool(name="p2", bufs=2))
        g.p2 = p2

        # ---- constants ----
        g.ones_r = cpool.tile([1, 128], F32, name="ones_r")   # K=1 bcast lhsT
        nc.vector.memset(g.ones_r[:], 1.0)
        g.ones128 = cpool.tile([128, 128], F16, name="ones128")
        nc.vector.memset(g.ones128[:], 1.0)
        g.eps_c = cpool.tile([128, 1], F32, name="eps_c")
        nc.vector.memset(g.eps_c[:], EPS)
        g.shift_c = cpool.tile([128, 1], F32, name="shift_c")
        nc.vector.memset(g.shift_c[:], -SHIFT)

        qw_sb = cpool.tile([1, EQ], F32, name="qw_sb")
        qb_sb = cpool.tile([1, EQ], F32, name="qb_sb")
        kw_sb = cpool.tile([1, HD], F32, name="kw_sb")
        kb_sb = cpool.tile([1, HD], F32, name="kb_sb")
        nc.sync.dma_start(qw_sb[:], g.qw_d.ap())
        nc.sync.dma_start(qb_sb[:], g.qb_d.ap())
        nc.sync.dma_start(kw_sb[:], g.kw_d.ap())
        nc.sync.dma_start(kb_sb[:], g.kb_d.ap())

        g.qwB = cpool.tile([128, HQ, 64, 2], F32, name="qwB")
        g.qbB = cpool.tile([128, HQ, 64, 2], F32, name="qbB")
        g.kwB = cpool.tile([128, 64, 2], F32, name="kwB")
        g.kbB = cpool.tile([128, 64, 2], F32, name="kbB")

        # persistent activations
        g.xq_raw = persist.tile([128, NT, HQ, 64, 2], BF16, name="xq_raw")
        g.xk_raw = persist.tile([128, NT, 64, 2], BF16, name="xk_raw")
        g.v_s = persist.tile([128, NT, HD], F16, name="v_s")
        g.stats_s = persist.tile([128, NT, 4], F32, name="stats_s")
        g.stats_g = persist.tile([128, NT, 4], F32, name="stats_g")
        g.qT_s = persist.tile([128, HQ, T], BF16, name="qT_s")
        g.kT_s = persist.tile([128, T], BF16, name="kT_s")

        g.mu_q = cpool.tile([128, NT], F32, name="mu_q")
        g.rstd_q = cpool.tile([128, NT], F32, name="rstd_q")
        g.mu_k = cpool.tile([128, NT], F32, name="mu_k")
        g.rstd_k = cpool.tile([128, NT], F32, name="rstd_k")
        g.tmp_a = cpool.tile([128, NT], F32, name="tmp_a")
        g.tmp_b = cpool.tile([128, NT], F32, name="tmp_b")

        def all_reduce_half(hb):
            ar_in = ardram.tile([128, ST, 4], F32, tag=f"ar_in{hb}")
            ar_out = ardram.tile([128, ST, 4], F32, tag=f"ar_out{hb}",
                                 addr_space="Shared")
            nc.gpsimd.dma_start(ar_in[:], g.stats_s[:, ts(hb, ST)])
            nc.gpsimd.collective_compute(
                "AllReduce", ALU.add,
                replica_groups=[list(range(NCORES))],
                ins=[ar_in.opt()], outs=[ar_out.opt()])
            nc.gpsimd.dma_start(g.stats_g[:, ts(hb, ST)], ar_out[:])

        # -------- phase 1: q/k/v projection + stats + AllReduces ----------
        with tc.tile_pool(name="p1w", bufs=1) as p1w, \
             tc.tile_pool(name="p1x", bufs=2) as p1x, \
             tc.tile_pool(name="p1s", bufs=2) as p1s, \
             tc.tile_pool(name="ps1", bufs=1, space="PSUM") as ps1:

            def load_xpair(tp):  # bf16 x for a PAIR of token tiles
                x_t = p1x.tile([128, ND, 256], BF16, tag="x_t", bufs=2)
                for j8 in range(0, ND, 8):
                    nc.sync.dma_start(x_t[:, ds(j8, 8), :],
                                      g.xT_d.ap()[:, ds(j8, 8), ts(tp, 256)])
                return x_t

            wq_s = p1w.tile([128, ND, EQ], BF16, name="wq_s")
            wkv_s = p1w.tile([128, ND, 2 * HD], BF16, name="wkv_s")
            # x pair 0 prefetched in parallel with the weight stream (weights
            # ride the gpsimd queue so they don't delay the x pipeline)
            nc.sync.dma_start(wq_s[:, ds(0, 8), :],
                              g.wqT_d.ap()[:, ds(0, 8), :])
            x_pre0 = load_xpair(0)
            nc.gpsimd.dma_start(wkv_s[:, ds(0, 8), :],
                                g.wkvT_d.ap()[:, ds(0, 8), :])
            for j8 in range(8, ND, 8):
                nc.gpsimd.dma_start(wq_s[:, ds(j8, 8), :],
                                    g.wqT_d.ap()[:, ds(j8, 8), :])
                nc.gpsimd.dma_start(wkv_s[:, ds(j8, 8), :],
                                    g.wkvT_d.ap()[:, ds(j8, 8), :])

            for bcsrc, bcdst, wid in ((qw_sb, g.qwB, EQ), (qb_sb, g.qbB, EQ),
                                      (kw_sb, g.kwB, HD), (kb_sb, g.kbB, HD)):
                ps_bc = ps1.tile([128, wid], F32, tag="psq", bufs=2)
                nc.tensor.matmul(ps_bc[:], lhsT=g.ones_r[:], rhs=bcsrc[:],
                                 start=True, stop=True)
                nc.scalar.copy(flat2(bcdst[:]), ps_bc[:])

            def qkv_tile(ti, x_pre=None):
                if ti % 2 == 0:
                    g.x_cur = x_pre if x_pre is not None else load_xpair(
                        ti // 2)
                x_t = g.x_cur
                tsl = ts(ti % 2, 128)
                psq = ps1.tile([128, EQ], F32, tag="psq", bufs=2)
                pskv = ps1.tile([128, 2 * HD], F32, tag="pskv", bufs=2)
                for j in range(ND):
                    nc.tensor.matmul(psq[:], lhsT=x_t[:, j, tsl],
                                     rhs=wq_s[:, j, :],
                                     start=(j == 0), stop=(j == ND - 1))
                    nc.tensor.matmul(pskv[:], lhsT=x_t[:, j, tsl],
                                     rhs=wkv_s[:, j, :],
                                     start=(j == 0), stop=(j == ND - 1))
                psk = pskv[:, 0:HD]
                nc.scalar.copy(flat2(g.xq_raw[:, ti]), psq[:])
                nc.scalar.copy(flat2(g.xk_raw[:, ti]), psk)
                nc.scalar.copy(g.v_s[:, ti, :], pskv[:, HD:2 * HD])
                scrap = p1s.tile([128, EQ], BF16, tag="scrap", bufs=2)
                nc.vector.tensor_reduce(out=g.stats_s[:, ti, 0:1],
                                        in_=psq[:], axis=AX.X, op=ALU.add)
                nc.scalar.activation(scrap[:], psq[:], AF.Square,
                                     accum_out=g.stats_s[:, ti, 1:2])
                scrapk = p1s.tile([128, HD], BF16, tag="scrapk", bufs=2)
                nc.vector.tensor_reduce(out=g.stats_s[:, ti, 2:3],
                                        in_=psk, axis=AX.X, op=ALU.add)
                nc.scalar.activation(scrapk[:], psk, AF.Square,
                                     accum_out=g.stats_s[:, ti, 3:4])

            qkv_tile(0, x_pre=x_pre0)
            for ti in range(1, ST):
                qkv_tile(ti)
            all_reduce_half(0)      # lands while tiles 16..31 project
            for ti in range(ST, 19):
                qkv_tile(ti)
            _postamble(nc, g, 0)
            # weave phase-2 (batch 0) DVE work into the phase-1 tail
            ph2_b0 = ([("k", ti) for ti in range(ST)]
                      + [("q", ti) for ti in range(8)])
            for i, ti in enumerate(range(19, NT)):
                qkv_tile(ti)
                for wh, t2 in ph2_b0[i * 24 // 13:(i + 1) * 24 // 13]:
                    _ph2_part(nc, g, t2, wh)
            all_reduce_half(1)      # lands during early attention b0

        # ---------------- phases 2+3+4, interleaved ----------------
        with tc.tile_pool(name="p34", bufs=1) as p34:
            g.oT_s = p34.tile([128, HQ, T], BF16, name="oT_s")
            g.woT_s = p34.tile([128, HQ, D], BF16, name="woT_s")
            nc.gpsimd.dma_start(g.woT_s[:], g.woT_d.ap())

            with tc.tile_pool(name="p3", bufs=1) as p3, \
                 tc.tile_pool(name="ps3", bufs=1, space="PSUM") as ps3:
                g.p3, g.ps3 = p3, ps3

                # fill[qb*HQ+h] emitted AFTER that q-block's body
                def mkfill():
                    return [[] for _ in range(NQB * HQ)]

                fill = mkfill()
                for i in range(4):              # q8..q11 feed qb2
                    fill[i] = [("q", 8 + i)]
                for i in range(4):              # q12..q15 feed qb3
                    fill[4 + i] = [("q", 12 + i), ("wo", i)]
                fill[8] = [("wo", 4), ("wo", 5)]
                fill[9] = [("wo", 6), ("wo", 7)]
                # stats-dependent parts only after AR half-1 has landed
                fill[10] = [("post2",), ("k", 16), ("k", 17)]
                fill[11] = [("k", 18), ("k", 19), ("k", 20), ("wo", 8)]
                fill[12] = [("k", 21), ("k", 22), ("k", 23), ("wo", 9)]
                fill[13] = [("k", 24), ("k", 25), ("q", 16), ("wo", 10)]
                fill[14] = [("k", 26), ("k", 27), ("k", 28), ("k", 29),
                            ("q", 17), ("wo", 11)]
                fill[15] = [("q", 18), ("q", 19), ("k", 30), ("k", 31)]
                _attn_batch(nc, g, 0, fill)
                fill = mkfill()
                fill[0] = [("q", 20), ("wo", 12), ("wo", 13)]
                fill[1] = [("q", 21), ("wo", 14), ("wo", 15)]
                fill[2] = [("q", 22)]
                fill[3] = [("q", 23)]
                for i in range(8):              # q24..q31 + wo16..23
                    fill[4 + i] = [("q", 24 + i), ("wo", 16 + i)]
                for i in range(4):
                    fill[12 + i] = [("wo", 24 + i)]
                _attn_batch(nc, g, 1, fill)
                for ti in range(28, NT):
                    _wo_tile(nc, g, ti)


def _postamble(nc, g, hb):
    """mu/rstd for one AllReduce half (token tiles hb*ST..hb*ST+ST-1)."""
    sl = ts(hb, ST)

    def stat(k):
        return g.stats_g[:, sl, k:k + 1].rearrange("p t s -> p (t s)")

    for (mu_t, rstd_t, s0, s1, e_full) in (
            (g.mu_q, g.rstd_q, 0, 1, D),
            (g.mu_k, g.rstd_k, 2, 3, KV * HD)):
        nc.vector.tensor_scalar_mul(mu_t[:, sl], stat(s0), 1.0 / e_full)
        nc.vector.tensor_scalar_mul(g.tmp_a[:, sl], stat(s1), 1.0 / e_full)
        nc.vector.tensor_mul(g.tmp_b[:, sl], mu_t[:, sl], mu_t[:, sl])
        nc.vector.tensor_sub(g.tmp_a[:, sl], g.tmp_a[:, sl], g.tmp_b[:, sl])
        nc.scalar.activation(g.tmp_b[:, sl], g.tmp_a[:, sl], AF.Sqrt,
                             bias=g.eps_c[:])
        nc.vector.reciprocal(rstd_t[:, sl], g.tmp_b[:, sl])


def _ph2_part(nc, g, ti, which):
    """LN apply + RoPE + DMA-XBAR transpose for q or k of one token tile."""
    p2 = g.p2
    cos_t = p2.tile([128, 64, 2], BF16, tag="cos", bufs=3)
    sin_t = p2.tile([128, 64, 2], BF16, tag="sin", bufs=3)
    nc.sync.dma_start(cos_t[:], g.cosk_d.ap()[ts(ti, 128)])
    nc.sync.dma_start(sin_t[:], g.sink_d.ap()[ts(ti, 128)])
    if which == "q":
        nh, raw, wB, bB = HQ, g.xq_raw[:, ti], g.qwB, g.qbB
        mu_t, rstd_t, tT = g.mu_q, g.rstd_q, g.qT_s
    else:
        nh, raw, wB, bB = 1, g.xk_raw[:, ti], g.kwB, g.kbB
        mu_t, rstd_t, tT = g.mu_k, g.rstd_k, g.kT_s

    shp = [128, nh, 64, 2] if nh > 1 else [128, 64, 2]
    xn_t = p2.tile(shp, BF16, tag=f"xn{which}", bufs=2)
    x2_t = p2.tile(shp, BF16, tag=f"x2{which}", bufs=2)
    rot_t = p2.tile(shp, BF16, tag=f"rot{which}", bufs=2)
    rp_t = p2.tile(shp, BF16, tag=f"rp{which}", bufs=2)

    # xn = (raw - mu) * rstd  (one fused DVE pass), then *w, +b
    nc.vector.tensor_scalar(out=xn_t[:], in0=raw,
                            scalar1=mu_t[:, ti:ti + 1],
                            scalar2=rstd_t[:, ti:ti + 1],
                            op0=ALU.subtract, op1=ALU.mult)
    nc.vector.tensor_mul(x2_t[:], xn_t[:], wB[:])
    nc.vector.tensor_add(x2_t[:], x2_t[:], bB[:])
    # rope: rp = x2*cos + swap(x2)*sin_signed
    if nh > 1:
        nc.vector.tensor_copy(rot_t[:, :, :, 0:1], x2_t[:, :, :, 1:2])
        nc.vector.tensor_copy(rot_t[:, :, :, 1:2], x2_t[:, :, :, 0:1])
        for h in range(nh):
            nc.vector.tensor_mul(xn_t[:, h], x2_t[:, h], cos_t[:])
            nc.vector.tensor_mul(rot_t[:, h], rot_t[:, h], sin_t[:])
        nc.vector.tensor_add(rp_t[:], xn_t[:], rot_t[:])
    else:
        nc.vector.tensor_copy(rot_t[:, :, 0:1], x2_t[:, :, 1:2])
        nc.vector.tensor_copy(rot_t[:, :, 1:2], x2_t[:, :, 0:1])
        nc.vector.tensor_mul(xn_t[:], x2_t[:], cos_t[:])
        nc.vector.tensor_mul(rot_t[:], rot_t[:], sin_t[:])
        nc.vector.tensor_add(rp_t[:], xn_t[:], rot_t[:])

    # [tok,hd] -> [hd,tok] via the DMA crossbar (sync hwdge queue)
    for h in range(nh):
        src = flat2(rp_t[:, h]) if nh > 1 else flat2(rp_t[:])
        dst = tT[:, h, ts(ti, 128)] if nh > 1 else tT[:, ts(ti, 128)]
        nc.sync.dma_start(dst, src, transpose=True)


def _attn_batch(nc, g, b, fill):
    """Attention for one batch, q-block outer / head inner. fill[qb*HQ+h]
    lists filler items emitted AFTER that q-block body: ("q"|"k", ti) for
    phase-2 parts, ("post2",) for the stats postamble, ("wo", ti) for an
    output-projection tile."""
    p3, ps3 = g.p3, g.ps3
    for qb in range(NQB):
        for h in range(HQ):
            q_ap = g.qT_s[:, h, ds(b * S + qb * 512, 512)]
            psV = ps3.tile([128, 512], F32, tag="psV", bufs=1)
            psBs = []
            lvl = [[], [], []]      # adder-tree partial tiles by level

            def mk_psB2():  # scores for a PAIR of k tiles into 2 psum banks
                t = ps3.tile([128, 2, 512], F32, tag="psB", bufs=2)
                for par in range(2):
                    nc.tensor.matmul(
                        t[:, par],
                        lhsT=g.kT_s[:, ds(b * S + (len(psBs) * 2 + par) * 128,
                                          128)],
                        rhs=q_ap, start=True, stop=True)
                psBs.append(t)

            def tree_add(level, a, bb):  # a/bb are ready APs
                t = p3.tile([128, 512], F16, tag=f"sum{level}",
                            bufs=(4 if level == 1 else 3))
                nc.vector.tensor_add(t[:], a, bb)
                lvl[level - 1].append(t)

            mk_psB2()
            mk_psB2()
            for kp in range(ST // 2):
                attnT = p3.tile([128, 2, 512], F16, tag="attnT", bufs=6)
                nc.scalar.activation(flat2(attnT[:]), flat2(psBs[kp][:]),
                                     AF.Exp, bias=g.shift_c[:])
                for par in range(2):
                    nc.tensor.matmul(psV[:],
                                     lhsT=g.v_s[:, b * ST + kp * 2 + par, :],
                                     rhs=attnT[:, par],
                                     start=(kp == 0 and par == 0),
                                     stop=(kp == ST // 2 - 1 and par == 1))
                if kp + 2 < ST // 2:
                    mk_psB2()
                # fp16 pairwise adder tree on DVE (2x rate), pipelined
                tree_add(1, attnT[:, 0], attnT[:, 1])
                for level in range(1, 3):
                    if len(lvl[level - 1]) == 2:
                        tree_add(level + 1, lvl[level - 1][0][:],
                                 lvl[level - 1][1][:])
                        lvl[level - 1] = []
            tree_add(3, lvl[2][0][:], lvl[2][1][:])  # final merge
            # combine + broadcast denominators across partitions in one MM
            psR = ps3.tile([128, 512], F32, tag="psR", bufs=1)
            nc.tensor.matmul(psR[:], lhsT=g.ones128[:], rhs=lvl[2][-1][:],
                             start=True, stop=True)
            recipB = p3.tile([128, 512], F32, tag="recipB", bufs=2)
            nc.vector.reciprocal_approx_fast(out=recipB[:], in_=psR[:])
            nc.vector.tensor_mul(g.oT_s[:, h, ds(b * S + qb * 512, 512)],
                                 psV[:], recipB[:])
            for item in fill[qb * HQ + h]:
                if item[0] == "post2":
                    _postamble(nc, g, 1)
                elif item[0] == "wo":
                    _wo_tile(nc, g, item[1])
                else:
                    _ph2_part(nc, g, item[1], item[0])


def _wo_tile(nc, g, ti):
    """Output projection for one 128-token tile (two 2048-wide halves)."""
    p3, ps3 = g.p3, g.ps3
    for qtr in range(4):
        outst = p3.tile([128, 2, 512], BF16, tag="outst", bufs=2)
        for nb in range(2):
            psO = ps3.tile([128, 512], F32, tag="psO", bufs=2)
            for h in range(HQ):
                nc.tensor.matmul(
                    psO[:], lhsT=g.oT_s[:, h, ts(ti, 128)],
                    rhs=g.woT_s[:, h, ds(qtr * 1024 + nb * 512, 512)],
                    start=(h == 0), stop=(h == HQ - 1))
            if nb == 0:
                nc.scalar.copy(outst[:, nb], psO[:])
            else:
                nc.vector.tensor_copy(outst[:, nb], psO[:])
        nc.sync.dma_start(g.out_d.ap()[ts(ti, 128), ds(qtr * 1024, 1024)],
                          flat2(outst[:]))


def _host_inputs(x, freqs_cis, wq, wk, wv, wo, q_norm_w, q_norm_b,
                 k_norm_w, k_norm_b):
    bf = ml_dtypes.bfloat16
    f32 = np.float32
    x = np.asarray(x, f32)
    freqs_cis = np.asarray(freqs_cis, f32)
    wq = np.asarray(wq, f32)
    wk = np.asarray(wk, f32)
    wv = np.asarray(wv, f32)
    wo = np.asarray(wo, f32)
    q_norm_w = np.asarray(q_norm_w, f32)
    q_norm_b = np.asarray(q_norm_b, f32)
    k_norm_w = np.asarray(k_norm_w, f32)
    k_norm_b = np.asarray(k_norm_b, f32)

    xf = np.ascontiguousarray(x.reshape(T, D))
    xT_r = np.ascontiguousarray(
        xf.T.reshape(ND, 128, T).transpose(1, 0, 2)).astype(bf)

    cos = freqs_cis[:, :, 0]          # [S, 64]
    sin = freqs_cis[:, :, 1]
    cos2 = np.concatenate([cos] * B, 0)   # [T, 64]
    sin2 = np.concatenate([sin] * B, 0)
    cosk = np.ascontiguousarray(np.stack([cos2, cos2], -1)).astype(bf)
    sink = np.ascontiguousarray(np.stack([-sin2, sin2], -1)).astype(bf)

    scale = 1.0 / np.sqrt(np.float32(HD))
    in_maps = []
    for c in range(NCORES):
        wq_c = wq[c * EQ:(c + 1) * EQ]           # [512, D]
        wk_c = wk[c * HD:(c + 1) * HD]           # [128, D]
        wv_c = wv[c * HD:(c + 1) * HD]
        wo_c = wo[:, c * EQ:(c + 1) * EQ]        # [D, 512]
        wqT_r = np.ascontiguousarray(
            wq_c.T.reshape(ND, 128, EQ).transpose(1, 0, 2)).astype(bf)
        wkT_r = np.ascontiguousarray(
            wk_c.T.reshape(ND, 128, HD).transpose(1, 0, 2)).astype(bf)
        wvT_r = np.ascontiguousarray(
            wv_c.T.reshape(ND, 128, HD).transpose(1, 0, 2)).astype(bf)
        wkvT_r = np.ascontiguousarray(
            np.concatenate([wkT_r, wvT_r], axis=2))
        woT_r = np.ascontiguousarray(
            wo_c.T.reshape(HQ, 128, D).transpose(1, 0, 2)).astype(bf)
        qw_c = (q_norm_w[c * EQ:(c + 1) * EQ] * scale).astype(f32).reshape(1, EQ)
        qb_c = (q_norm_b[c * EQ:(c + 1) * EQ] * scale).astype(f32).reshape(1, EQ)
        kw_c = k_norm_w[c * HD:(c + 1) * HD].astype(f32).reshape(1, HD)
        kb_c = k_norm_b[c * HD:(c + 1) * HD].astype(f32).reshape(1, HD)
        in_maps.append({
            "xT": xT_r, "wqT": wqT_r, "wkvT": wkvT_r, "woT": woT_r,
            "cosk": cosk, "sink": sink,
            "qw": qw_c, "qb": qb_c, "kw": kw_c, "kb": kb_c,
        })
    return in_maps


def _run_profiled(nc, in_maps):
    """bass2jax execute wrapped in an NRT profile capture; returns
    (results, max exec_time_ns across cores, trace_dir)."""
    import ctypes
    import glob
    import tempfile

    import jax
    from concourse import bass2jax
    import gauge.profiler
    from concourse.bass_utils import FishPath

    lib = ctypes.CDLL("/opt/axon/libaxon_pjrt.so")
    if not hasattr(lib, "axon_start_nrt_profile"):
        results = bass2jax.run_bass_via_pjrt(nc, in_maps, n_cores=NCORES)
        return results, None, None
    lib.axon_start_nrt_profile.argtypes = [ctypes.POINTER(ctypes.c_int64),
                                           ctypes.c_size_t]
    lib.axon_start_nrt_profile.restype = ctypes.c_int64
    lib.axon_stop_nrt_profile.argtypes = [ctypes.c_char_p]
    lib.axon_stop_nrt_profile.restype = ctypes.c_int64

    jax.devices()
    # warm-up execution: loads the NEFF and aligns core dispatch so the
    # profiled run isn't polluted by first-run start skew
    bass2jax.run_bass_via_pjrt(nc, in_maps, n_cores=NCORES)
    neff_dir = tempfile.mkdtemp(prefix="bassprof_")
    rc = lib.axon_start_nrt_profile(None, 0)
    if rc != 0:
        raise RuntimeError(f"axon_start_nrt_profile rc={rc}")
    try:
        results = bass2jax.run_bass_via_pjrt(nc, in_maps, n_cores=NCORES)
    finally:
        n = lib.axon_stop_nrt_profile(neff_dir.encode())
        print(f"profile: {n} ntff file(s) in {neff_dir}")
    ntffs = glob.glob(neff_dir + "/*_body*.ntff")
    if not ntffs:
        return results, None, None
    profile = gauge.profiler.Profile(
        profile_path=FishPath(neff_dir), kernel_dev_mode=True,
        profile_on_exit=False, bass_kernel=nc.m,
        offline_processing=True, fname="*_body*")
    exec_ns = None
    try:
        prs = profile.to_perfetto(model_index=list(range(NCORES)))
        times = [pr.exec_time_ns for pr in prs if pr.exec_time_ns]
        exec_ns = max(times) if times else None
    except Exception as e:  # profile parse best-effort
        print("profile parse failed:", e)
    return results, exec_ns, neff_dir


def kernel(x, freqs_cis, wq, wk, wv, wo, q_norm_w, q_norm_b,
           k_norm_w, k_norm_b):
    global LAST_EXEC_NS, LAST_TRACE_DIR
    nc = _build()
    in_maps = _host_inputs(x, freqs_cis, wq, wk, wv, wo,
                           q_norm_w, q_norm_b, k_norm_w, k_norm_b)
    if PROFILE:
        results, LAST_EXEC_NS, LAST_TRACE_DIR = _run_profiled(nc, in_maps)
    else:
        res = bass_utils.run_bass_kernel_spmd(
            nc, in_maps, core_ids=list(range(NCORES)))
        results = res.results
        LAST_EXEC_NS = res.exec_time_ns
    acc = np.zeros((T, D), np.float32)
    for r in results:
        acc += np.asarray(r["out"], np.float32)
    return acc.reshape(B, S, D)


# revision 17
# speedup vs baseline: 1.0315x; 1.0315x over previous
"""GQA attention block (B=2,S=2048,D=4096,H=32,KV=8,HD=128) on 8 TRN2 NeuronCores.

Sharding: 8-way tensor parallel over heads. Core c owns kv-head c and q-heads
4c..4c+3 (wq/wk/wv column-sharded, wo row-sharded). The full-width Q/K
layernorms need cross-core mean/var, done with one tiny (64KB) on-device
AllReduce of per-token partial sums. Each core emits a partial [T,D] output
(its wo row-slice contribution); the host sums the 8 partials.

Device pipeline per core (all matmuls bf16/fp16, f32 accumulation):
  1. Q projection (N=512) interleaved with a fused [wk|wv] projection (N=256)
     so every x-chunk stationary is loaded once and reused by both streams.
     x arrives in 256-token pair loads (512B runs, full DMA rate) on the sync
     queue; weights stream on the gpsimd queue. LN partial stats per tile;
     two AllReduce halves overlap compute.
  2. LN apply + RoPE (shared cos/sin across q heads); transposes to [hd,t]
     via the DMA XBAR (scalar queue) instead of the PE array.
  3. Attention per (b,h): scores^T = k_tile^T q, exp on ACT into fp16,
     attn@V with fp16 v stationary. Softmax sums via an fp16 pairwise adder
     tree on DVE (2x rate) plus one all-ones matmul that both combines and
     broadcasts the denominator; fast approximate reciprocal normalizes in
     the psum->sbuf copyback.
  4. Output projection tiles woven into the attention fill slots (keeps PE
     busy while ACT runs exp, removes the tail); psum->sbuf casts on ACT.
"""

from contextlib import ExitStack

import numpy as np
import ml_dtypes

import concourse.bass as bass
import concourse.mybir as mybir
import concourse.tile as tile
from concourse import bacc
from concourse import bass_utils
from concourse.bass import ts, ds
from concourse.masks import make_identity

BF16 = mybir.dt.bfloat16
F16 = mybir.dt.float16
F32 = mybir.dt.float32
AF = mybir.ActivationFunctionType
ALU = mybir.AluOpType
AX = mybir.AxisListType

B, S, D = 2, 2048, 4096
T = B * S                 # 4096 tokens
H, KV, HD = 32, 8, 128
NCORES = 8
HQ = H // NCORES          # 4 q heads per core
EQ = HQ * HD              # 512
NT = T // 128             # 32 token tiles
ND = D // 128             # 32 contraction chunks
ST = S // 128             # 16 seq tiles per batch
NQB = S // 512            # 4 q-blocks per (b,h)
EPS = 1e-5
SHIFT = 4.0               # softmax shift: exp(s-4) stays in fp16 normal range

PROFILE = False
LAST_EXEC_NS = None
LAST_TRACE_DIR = None
_CACHE = {}


def flat2(ap):  # flatten all free dims -> [P, prod(free)]
    n = len(ap.shape)
    if n == 2:
        return ap
    names = " ".join(f"d{i}" for i in range(n - 1))
    return ap.rearrange(f"p {names} -> p ({names})")


class _Ctx:
    pass


def _build():
    if "nc" in _CACHE:
        return _CACHE["nc"]
    nc = bacc.Bacc("TRN2", target_bir_lowering=False, debug=False,
                   num_devices=NCORES)

    g = _Ctx()
    g.xT_d = nc.dram_tensor("xT", [128, ND, T], BF16, kind="ExternalInput")
    g.wqT_d = nc.dram_tensor("wqT", [128, ND, EQ], BF16, kind="ExternalInput")
    g.wkvT_d = nc.dram_tensor("wkvT", [128, ND, 2 * HD], BF16,
                              kind="ExternalInput")
    g.woT_d = nc.dram_tensor("woT", [128, HQ, D], BF16, kind="ExternalInput")
    g.cosk_d = nc.dram_tensor("cosk", [T, 64, 2], BF16, kind="ExternalInput")
    g.sink_d = nc.dram_tensor("sink", [T, 64, 2], BF16, kind="ExternalInput")
    g.qw_d = nc.dram_tensor("qw", [1, EQ], F32, kind="ExternalInput")
    g.qb_d = nc.dram_tensor("qb", [1, EQ], F32, kind="ExternalInput")
    g.kw_d = nc.dram_tensor("kw", [1, HD], F32, kind="ExternalInput")
    g.kb_d = nc.dram_tensor("kb", [1, HD], F32, kind="ExternalInput")
    g.out_d = nc.dram_tensor("out", [T, D], BF16, kind="ExternalOutput")

    with tile.TileContext(nc) as tc:
        _emit(nc, tc, g)
    nc.compile()
    _CACHE["nc"] = nc
    return nc


def _emit(nc, tc, g):
    ctx = ExitStack()
    with ctx:
        cpool = ctx.enter_context(tc.tile_pool(name="cpool", bufs=1))
        persist = ctx.enter_context(tc.tile_pool(name="persist", bufs=1))
        ardram = ctx.enter_context(
            tc.tile_pool(name="ardram", bufs=1, space="DRAM"))
        p2 = ctx.enter_context(tc.tile_pool(name="p2", bufs=2))
        g.p2 = p2

        # ---- constants ----
        g.ident = cpool.tile([128, 128], BF16, name="ident")
        make_identity(nc, g.ident[:])
        g.ones_r = cpool.tile([1, 128], F32, name="ones_r")   # K=1 bcast lhsT
        nc.vector.memset(g.ones_r[:], 1.0)
        g.ones128 = cpool.tile([128, 128], F16, name="ones128")
        nc.vector.memset(g.ones128[:], 1.0)
        g.eps_c = cpool.tile([128, 1], F32, name="eps_c")
        nc.vector.memset(g.eps_c[:], EPS)
        g.shift_c = cpool.tile([128, 1], F32, name="shift_c")
        nc.vector.memset(g.shift_c[:], -SHIFT)

        qw_sb = cpool.tile([1, EQ], F32, name="qw_sb")
        qb_sb = cpool.tile([1, EQ], F32, name="qb_sb")
        kw_sb = cpool.tile([1, HD], F32, name="kw_sb")
        kb_sb = cpool.tile([1, HD], F32, name="kb_sb")
        nc.sync.dma_start(qw_sb[:], g.qw_d.ap())
        nc.sync.dma_start(qb_sb[:], g.qb_d.ap())
        nc.sync.dma_start(kw_sb[:], g.kw_d.ap())
        nc.sync.dma_start(kb_sb[:], g.kb_d.ap())

        g.qwB = cpool.tile([128, HQ, 64, 2], F32, name="qwB")
        g.qbB = cpool.tile([128, HQ, 64, 2], F32, name="qbB")
        g.kwB = cpool.tile([128, 64, 2], F32, name="kwB")
        g.kbB = cpool.tile([128, 64, 2], F32, name="kbB")

        # persistent activations
        g.xq_raw = persist.tile([128, NT, HQ, 64, 2], BF16, name="xq_raw")
        g.xk_raw = persist.tile([128, NT, 64, 2], BF16, name="xk_raw")
        g.v_s = persist.tile([128, NT, HD], F16, name="v_s")
        g.stats_s = persist.tile([128, NT, 4], F32, name="stats_s")
        g.stats_g = persist.tile([128, NT, 4], F32, name="stats_g")
        g.qT_s = persist.tile([128, HQ, T], BF16, name="qT_s")
        g.kT_s = persist.tile([128, T], BF16, name="kT_s")

        g.mu_q = cpool.tile([128, NT], F32, name="mu_q")
        g.rstd_q = cpool.tile([128, NT], F32, name="rstd_q")
        g.mu_k = cpool.tile([128, NT], F32, name="mu_k")
        g.rstd_k = cpool.tile([128, NT], F32, name="rstd_k")
        g.tmp_a = cpool.tile([128, NT], F32, name="tmp_a")
        g.tmp_b = cpool.tile([128, NT], F32, name="tmp_b")

        def all_reduce_half(hb):
            ar_in = ardram.tile([128, ST, 4], F32, tag=f"ar_in{hb}")
            ar_out = ardram.tile([128, ST, 4], F32, tag=f"ar_out{hb}",
                                 addr_space="Shared")
            nc.gpsimd.dma_start(ar_in[:], g.stats_s[:, ts(hb, ST)])
            nc.gpsimd.collective_compute(
                "AllReduce", ALU.add,
                replica_groups=[list(range(NCORES))],
                ins=[ar_in.opt()], outs=[ar_out.opt()])
            nc.gpsimd.dma_start(g.stats_g[:, ts(hb, ST)], ar_out[:])

        # -------- phase 1: q/k/v projection + stats + AllReduces ----------
        with tc.tile_pool(name="p1w", bufs=1) as p1w, \
             tc.tile_pool(name="p1x", bufs=2) as p1x, \
             tc.tile_pool(name="p1s", bufs=2) as p1s, \
             tc.tile_pool(name="ps1", bufs=1, space="PSUM") as ps1:

            g.tp_pool = ps1

            def load_xpair(tp):  # bf16 x for a PAIR of token tiles
                x_t = p1x.tile([128, ND, 256], BF16, tag="x_t", bufs=2)
                for j8 in range(0, ND, 8):
                    nc.sync.dma_start(x_t[:, ds(j8, 8), :],
                                      g.xT_d.ap()[:, ds(j8, 8), ts(tp, 256)])
                return x_t

            wq_s = p1w.tile([128, ND, EQ], BF16, name="wq_s")
            wkv_s = p1w.tile([128, ND, 2 * HD], BF16, name="wkv_s")
            # x pair 0 prefetched in parallel with the weight stream (weights
            # ride the gpsimd queue so they don't delay the x pipeline)
            nc.sync.dma_start(wq_s[:, ds(0, 8), :],
                              g.wqT_d.ap()[:, ds(0, 8), :])
            nc.sync.dma_start(wkv_s[:, ds(0, 8), :],
                              g.wkvT_d.ap()[:, ds(0, 8), :])
            x_pre0 = load_xpair(0)
            for j8 in range(8, ND, 8):
                nc.gpsimd.dma_start(wq_s[:, ds(j8, 8), :],
                                    g.wqT_d.ap()[:, ds(j8, 8), :])
                nc.gpsimd.dma_start(wkv_s[:, ds(j8, 8), :],
                                    g.wkvT_d.ap()[:, ds(j8, 8), :])

            for bcsrc, bcdst, wid in ((qw_sb, g.qwB, EQ), (qb_sb, g.qbB, EQ),
                                      (kw_sb, g.kwB, HD), (kb_sb, g.kbB, HD)):
                ps_bc = ps1.tile([128, wid], F32, tag="psq", bufs=2)
                nc.tensor.matmul(ps_bc[:], lhsT=g.ones_r[:], rhs=bcsrc[:],
                                 start=True, stop=True)
                nc.scalar.copy(flat2(bcdst[:]), ps_bc[:])

            def qkv_tile(ti, x_pre=None):
                if ti % 2 == 0:
                    g.x_cur = x_pre if x_pre is not None else load_xpair(
                        ti // 2)
                x_t = g.x_cur
                tsl = ts(ti % 2, 128)
                psq = ps1.tile([128, EQ], F32, tag="psq", bufs=2)
                pskv = ps1.tile([128, 2 * HD], F32, tag="pskv", bufs=2)
                for j in range(ND):
                    nc.tensor.matmul(psq[:], lhsT=x_t[:, j, tsl],
                                     rhs=wq_s[:, j, :],
                                     start=(j == 0), stop=(j == ND - 1))
                    nc.tensor.matmul(pskv[:], lhsT=x_t[:, j, tsl],
                                     rhs=wkv_s[:, j, :],
                                     start=(j == 0), stop=(j == ND - 1))
                psk = pskv[:, 0:HD]
                nc.scalar.copy(flat2(g.xq_raw[:, ti]), psq[:])
                nc.scalar.copy(flat2(g.xk_raw[:, ti]), psk)
                nc.scalar.copy(g.v_s[:, ti, :], pskv[:, HD:2 * HD])
                scrap = p1s.tile([128, EQ], BF16, tag="scrap", bufs=2)
                nc.vector.tensor_reduce(out=g.stats_s[:, ti, 0:1],
                                        in_=psq[:], axis=AX.X, op=ALU.add)
                nc.scalar.activation(scrap[:], psq[:], AF.Square,
                                     accum_out=g.stats_s[:, ti, 1:2])
                scrapk = p1s.tile([128, HD], BF16, tag="scrapk", bufs=2)
                nc.vector.tensor_reduce(out=g.stats_s[:, ti, 2:3],
                                        in_=psk, axis=AX.X, op=ALU.add)
                nc.scalar.activation(scrapk[:], psk, AF.Square,
                                     accum_out=g.stats_s[:, ti, 3:4])

            qkv_tile(0, x_pre=x_pre0)
            for ti in range(1, ST):
                qkv_tile(ti)
            all_reduce_half(0)      # lands while tiles 16..31 project
            for ti in range(ST, 19):
                qkv_tile(ti)
            _postamble(nc, g, 0)
            # weave phase-2 (batch 0) DVE work into the phase-1 tail
            ph2_b0 = ([("q", ti) for ti in range(4)]
                      + [("k", ti) for ti in range(ST)]
                      + [("q", ti) for ti in range(4, 8)])
            for i, ti in enumerate(range(19, NT)):
                qkv_tile(ti)
                for wh, t2 in ph2_b0[i * 24 // 13:(i + 1) * 24 // 13]:
                    _ph2_part(nc, g, t2, wh, pe_tp=True)
            all_reduce_half(1)      # lands during early attention b0

        # ---------------- phases 2+3+4, interleaved ----------------
        with tc.tile_pool(name="p34", bufs=1) as p34:
            g.oT_s = p34.tile([128, HQ, T], BF16, name="oT_s")
            g.woT_s = p34.tile([128, HQ, D], BF16, name="woT_s")
            nc.gpsimd.dma_start(g.woT_s[:], g.woT_d.ap())

            with tc.tile_pool(name="p3", bufs=1) as p3, \
                 tc.tile_pool(name="ps3", bufs=1, space="PSUM") as ps3:
                g.p3, g.ps3 = p3, ps3

                # fill[qb*HQ+h] emitted AFTER that q-block's body
                def mkfill():
                    return [[] for _ in range(NQB * HQ)]

                fill = mkfill()
                for i in range(4):              # q8..q11 feed qb2
                    fill[i] = [("q", 8 + i)]
                # post2 lands after AR half-1 (~slot 3); b1-q parts early
                fill[4] = [("post2",), ("q", 16), ("wo", 0)]
                fill[5] = [("q", 17), ("wo", 1)]
                fill[6] = [("q", 18), ("wo", 2)]
                fill[7] = [("q", 19), ("wo", 3)]
                fill[8] = [("q", 12), ("q", 13), ("wo", 4)]
                fill[9] = [("q", 14), ("q", 15), ("wo", 5)]
                fill[10] = [("k", 16), ("k", 17), ("k", 18), ("k", 19),
                            ("wo", 6)]
                fill[11] = [("k", 20), ("k", 21), ("k", 22), ("k", 23),
                            ("wo", 7)]
                fill[12] = [("k", 24), ("k", 25), ("k", 26), ("wo", 8)]
                fill[13] = [("k", 27), ("k", 28), ("k", 29), ("wo", 9)]
                fill[14] = [("k", 30), ("k", 31), ("wo", 10), ("wo", 11)]
                fill[15] = []
                _attn_batch(nc, g, 0, fill)
                fill = mkfill()
                fill[0] = [("q", 20), ("wo", 12), ("wo", 13)]
                fill[1] = [("q", 21), ("wo", 14), ("wo", 15)]
                fill[2] = [("q", 22)]
                fill[3] = [("q", 23)]
                for i in range(8):              # q24..q31 + wo16..23
                    fill[4 + i] = [("q", 24 + i), ("wo", 16 + i)]
                for i in range(4):
                    fill[12 + i] = [("wo", 24 + i)]
                _attn_batch(nc, g, 1, fill)
                for ti in range(28, NT):
                    _wo_tile(nc, g, ti)


def _postamble(nc, g, hb):
    """mu/rstd for one AllReduce half (token tiles hb*ST..hb*ST+ST-1)."""
    sl = ts(hb, ST)

    def stat(k):
        return g.stats_g[:, sl, k:k + 1].rearrange("p t s -> p (t s)")

    for (mu_t, rstd_t, s0, s1, e_full) in (
            (g.mu_q, g.rstd_q, 0, 1, D),
            (g.mu_k, g.rstd_k, 2, 3, KV * HD)):
        nc.vector.tensor_scalar_mul(mu_t[:, sl], stat(s0), 1.0 / e_full)
        nc.vector.tensor_scalar_mul(g.tmp_a[:, sl], stat(s1), 1.0 / e_full)
        nc.vector.tensor_mul(g.tmp_b[:, sl], mu_t[:, sl], mu_t[:, sl])
        nc.vector.tensor_sub(g.tmp_a[:, sl], g.tmp_a[:, sl], g.tmp_b[:, sl])
        nc.scalar.activation(g.tmp_b[:, sl], g.tmp_a[:, sl], AF.Sqrt,
                             bias=g.eps_c[:])
        nc.vector.reciprocal(rstd_t[:, sl], g.tmp_b[:, sl])


def _ph2_part(nc, g, ti, which, pe_tp=False):
    """LN apply + RoPE + DMA-XBAR transpose for q or k of one token tile."""
    p2 = g.p2
    cos_t = p2.tile([128, 64, 2], BF16, tag="cos", bufs=3)
    sin_t = p2.tile([128, 64, 2], BF16, tag="sin", bufs=3)
    nc.sync.dma_start(cos_t[:], g.cosk_d.ap()[ts(ti, 128)])
    nc.sync.dma_start(sin_t[:], g.sink_d.ap()[ts(ti, 128)])
    if which == "q":
        nh, raw, wB, bB = HQ, g.xq_raw[:, ti], g.qwB, g.qbB
        mu_t, rstd_t, tT = g.mu_q, g.rstd_q, g.qT_s
    else:
        nh, raw, wB, bB = 1, g.xk_raw[:, ti], g.kwB, g.kbB
        mu_t, rstd_t, tT = g.mu_k, g.rstd_k, g.kT_s

    shp = [128, nh, 64, 2] if nh > 1 else [128, 64, 2]
    xn_t = p2.tile(shp, BF16, tag=f"xn{which}", bufs=2)
    x2_t = p2.tile(shp, BF16, tag=f"x2{which}", bufs=2)
    rot_t = p2.tile(shp, BF16, tag=f"rot{which}", bufs=2)
    rp_t = p2.tile(shp, BF16, tag=f"rp{which}", bufs=2)

    # xn = (raw - mu) * rstd  (one fused DVE pass), then *w, +b
    nc.vector.tensor_scalar(out=xn_t[:], in0=raw,
                            scalar1=mu_t[:, ti:ti + 1],
                            scalar2=rstd_t[:, ti:ti + 1],
                            op0=ALU.subtract, op1=ALU.mult)
    nc.vector.tensor_mul(x2_t[:], xn_t[:], wB[:])
    nc.vector.tensor_add(x2_t[:], x2_t[:], bB[:])
    # rope: rp = x2*cos + swap(x2)*sin_signed
    if nh > 1:
        nc.vector.tensor_copy(rot_t[:, :, :, 0:1], x2_t[:, :, :, 1:2])
        nc.vector.tensor_copy(rot_t[:, :, :, 1:2], x2_t[:, :, :, 0:1])
        for h in range(nh):
            nc.vector.tensor_mul(xn_t[:, h], x2_t[:, h], cos_t[:])
            nc.vector.tensor_mul(rot_t[:, h], rot_t[:, h], sin_t[:])
        nc.vector.tensor_add(rp_t[:], xn_t[:], rot_t[:])
    else:
        nc.vector.tensor_copy(rot_t[:, :, 0:1], x2_t[:, :, 1:2])
        nc.vector.tensor_copy(rot_t[:, :, 1:2], x2_t[:, :, 0:1])
        nc.vector.tensor_mul(xn_t[:], x2_t[:], cos_t[:])
        nc.vector.tensor_mul(rot_t[:], rot_t[:], sin_t[:])
        nc.vector.tensor_add(rp_t[:], xn_t[:], rot_t[:])

    # [tok,hd] -> [hd,tok]: PE transpose (low latency, phase-1 weave) or
    # DMA crossbar on the sync queue (off-PE, attention fills)
    for h in range(nh):
        src = rp_t[:, h] if nh > 1 else rp_t[:]
        dst = tT[:, h, ts(ti, 128)] if nh > 1 else tT[:, ts(ti, 128)]
        if pe_tp:
            tp_ps = g.tp_pool.tile([128, 128], BF16, tag="tp", bufs=2)
            nc.tensor.transpose(tp_ps[:], src, g.ident[:])
            nc.vector.tensor_copy(dst, tp_ps[:])
        else:
            nc.sync.dma_start(dst, flat2(src), transpose=True)


def _attn_batch(nc, g, b, fill):
    """Attention for one batch, q-block outer / head inner. fill[qb*HQ+h]
    lists filler items emitted AFTER that q-block body: ("q"|"k", ti) for
    phase-2 parts, ("post2",) for the stats postamble, ("wo", ti) for an
    output-projection tile."""
    p3, ps3 = g.p3, g.ps3
    for qb in range(NQB):
        for h in range(HQ):
            q_ap = g.qT_s[:, h, ds(b * S + qb * 512, 512)]
            psV = ps3.tile([128, 512], F32, tag="psV", bufs=1)
            psBs = []
            lvl = [[], [], []]      # adder-tree partial tiles by level

            def mk_psB2():  # scores for a PAIR of k tiles into 2 psum banks
                t = ps3.tile([128, 2, 512], F32, tag="psB", bufs=2)
                for par in range(2):
                    nc.tensor.matmul(
                        t[:, par],
                        lhsT=g.kT_s[:, ds(b * S + (len(psBs) * 2 + par) * 128,
                                          128)],
                        rhs=q_ap, start=True, stop=True)
                psBs.append(t)

            def tree_add(level, a, bb):  # a/bb are ready APs
                t = p3.tile([128, 512], F16, tag=f"sum{level}",
                            bufs=(4 if level == 1 else 3))
                nc.vector.tensor_add(t[:], a, bb)
                lvl[level - 1].append(t)

            mk_psB2()
            mk_psB2()
            for kp in range(ST // 2):
                attnT = p3.tile([128, 2, 512], F16, tag="attnT", bufs=6)
                nc.scalar.activation(flat2(attnT[:]), flat2(psBs[kp][:]),
                                     AF.Exp, bias=g.shift_c[:])
                for par in range(2):
                    nc.tensor.matmul(psV[:],
                                     lhsT=g.v_s[:, b * ST + kp * 2 + par, :],
                                     rhs=attnT[:, par],
                                     start=(kp == 0 and par == 0),
                                     stop=(kp == ST // 2 - 1 and par == 1))
                if kp + 2 < ST // 2:
                    mk_psB2()
                # fp16 pairwise adder tree on DVE (2x rate), pipelined
                tree_add(1, attnT[:, 0], attnT[:, 1])
                for level in range(1, 3):
                    if len(lvl[level - 1]) == 2:
                        tree_add(level + 1, lvl[level - 1][0][:],
                                 lvl[level - 1][1][:])
                        lvl[level - 1] = []
            tree_add(3, lvl[2][0][:], lvl[2][1][:])  # final merge
            # combine + broadcast denominators across partitions in one MM
            psR = ps3.tile([128, 512], F32, tag="psR", bufs=1)
            nc.tensor.matmul(psR[:], lhsT=g.ones128[:], rhs=lvl[2][-1][:],
                             start=True, stop=True)
            recipB = p3.tile([128, 512], F32, tag="recipB", bufs=2)
            nc.vector.reciprocal_approx_fast(out=recipB[:], in_=psR[:])
            nc.vector.tensor_mul(g.oT_s[:, h, ds(b * S + qb * 512, 512)],
                                 psV[:], recipB[:])
            for item in fill[qb * HQ + h]:
                if item[0] == "post2":
                    _postamble(nc, g, 1)
                elif item[0] == "wo":
                    _wo_tile(nc, g, item[1])
                else:
                    _ph2_part(nc, g, item[1], item[0])


def _wo_tile(nc, g, ti):
    """Output projection for one 128-token tile (two 2048-wide halves)."""
    p3, ps3 = g.p3, g.ps3
    for qtr in range(4):
        outst = p3.tile([128, 2, 512], BF16, tag="outst", bufs=2)
        for nb in range(2):
            psO = ps3.tile([128, 512], F32, tag="psO", bufs=2)
            for h in range(HQ):
                nc.tensor.matmul(
                    psO[:], lhsT=g.oT_s[:, h, ts(ti, 128)],
                    rhs=g.woT_s[:, h, ds(qtr * 1024 + nb * 512, 512)],
                    start=(h == 0), stop=(h == HQ - 1))
            if nb == 0:
                nc.scalar.copy(outst[:, nb], psO[:])
            else:
                nc.vector.tensor_copy(outst[:, nb], psO[:])
        nc.sync.dma_start(g.out_d.ap()[ts(ti, 128), ds(qtr * 1024, 1024)],
                          flat2(outst[:]))


def _host_inputs(x, freqs_cis, wq, wk, wv, wo, q_norm_w, q_norm_b,
                 k_norm_w, k_norm_b):
    bf = ml_dtypes.bfloat16
    f32 = np.float32
    x = np.asarray(x, f32)
    freqs_cis = np.asarray(freqs_cis, f32)
    wq = np.asarray(wq, f32)
    wk = np.asarray(wk, f32)
    wv = np.asarray(wv, f32)
    wo = np.asarray(wo, f32)
    q_norm_w = np.asarray(q_norm_w, f32)
    q_norm_b = np.asarray(q_norm_b, f32)
    k_norm_w = np.asarray(k_norm_w, f32)
    k_norm_b = np.asarray(k_norm_b, f32)

    xf = np.ascontiguousarray(x.reshape(T, D))
    xT_r = np.ascontiguousarray(
        xf.T.reshape(ND, 128, T).transpose(1, 0, 2)).astype(bf)

    cos = freqs_cis[:, :, 0]          # [S, 64]
    sin = freqs_cis[:, :, 1]
    cos2 = np.concatenate([cos] * B, 0)   # [T, 64]
    sin2 = np.concatenate([sin] * B, 0)
    cosk = np.ascontiguousarray(np.stack([cos2, cos2], -1)).astype(bf)
    sink = np.ascontiguousarray(np.stack([-sin2, sin2], -1)).astype(bf)

    scale = 1.0 / np.sqrt(np.float32(HD))
    in_maps = []
    for c in range(NCORES):
        wq_c = wq[c * EQ:(c + 1) * EQ]           # [512, D]
        wk_c = wk[c * HD:(c + 1) * HD]           # [128, D]
        wv_c = wv[c * HD:(c + 1) * HD]
        wo_c = wo[:, c * EQ:(c + 1) * EQ]        # [D, 512]
        wqT_r = np.ascontiguousarray(
            wq_c.T.reshape(ND, 128, EQ).transpose(1, 0, 2)).astype(bf)
        wkT_r = np.ascontiguousarray(
            wk_c.T.reshape(ND, 128, HD).transpose(1, 0, 2)).astype(bf)
        wvT_r = np.ascontiguousarray(
            wv_c.T.reshape(ND, 128, HD).transpose(1, 0, 2)).astype(bf)
        wkvT_r = np.ascontiguousarray(
            np.concatenate([wkT_r, wvT_r], axis=2))
        woT_r = np.ascontiguousarray(
            wo_c.T.reshape(HQ, 128, D).transpose(1, 0, 2)).astype(bf)
        qw_c = (q_norm_w[c * EQ:(c + 1) * EQ] * scale).astype(f32).reshape(1, EQ)
        qb_c = (q_norm_b[c * EQ:(c + 1) * EQ] * scale).astype(f32).reshape(1, EQ)
        kw_c = k_norm_w[c * HD:(c + 1) * HD].astype(f32).reshape(1, HD)
        kb_c = k_norm_b[c * HD:(c + 1) * HD].astype(f32).reshape(1, HD)
        in_maps.append({
            "xT": xT_r, "wqT": wqT_r, "wkvT": wkvT_r, "woT": woT_r,
            "cosk": cosk, "sink": sink,
            "qw": qw_c, "qb": qb_c, "kw": kw_c, "kb": kb_c,
        })
    return in_maps


def _run_profiled(nc, in_maps):
    """bass2jax execute wrapped in an NRT profile capture; returns
    (results, max exec_time_ns across cores, trace_dir)."""
    import ctypes
    import glob
    import tempfile

    import jax
    from concourse import bass2jax
    import gauge.profiler
    from concourse.bass_utils import FishPath

    lib = ctypes.CDLL("/opt/axon/libaxon_pjrt.so")
    if not hasattr(lib, "axon_start_nrt_profile"):
        results = bass2jax.run_bass_via_pjrt(nc, in_maps, n_cores=NCORES)
        return results, None, None
    lib.axon_start_nrt_profile.argtypes = [ctypes.POINTER(ctypes.c_int64),
                                           ctypes.c_size_t]
    lib.axon_start_nrt_profile.restype = ctypes.c_int64
    lib.axon_stop_nrt_profile.argtypes = [ctypes.c_char_p]
    lib.axon_stop_nrt_profile.restype = ctypes.c_int64

    jax.devices()
    # warm-up execution: loads the NEFF and aligns core dispatch so the
    # profiled run isn't polluted by first-run start skew
    bass2jax.run_bass_via_pjrt(nc, in_maps, n_cores=NCORES)
    neff_dir = tempfile.mkdtemp(prefix="bassprof_")
    rc = lib.axon_start_nrt_profile(None, 0)
    if rc != 0:
        raise RuntimeError(f"axon_start_nrt_profile rc={rc}")
    try:
        results = bass2jax.run_bass_via_pjrt(nc, in_maps, n_cores=NCORES)
    finally:
        n = lib.axon_stop_nrt_profile(neff_dir.encode())
        print(f"profile: {n} ntff file(s) in {neff_dir}")
    ntffs = glob.glob(neff_dir + "/*_body*.ntff")
    if not ntffs:
        return results, None, None
    profile = gauge.profiler.Profile(
        profile_path=FishPath(neff_dir), kernel_dev_mode=True,
        profile_on_exit=False, bass_kernel=nc.m,
        offline_processing=True, fname="*_body*")
    exec_ns = None
    try:
        prs = profile.to_perfetto(model_index=list(range(NCORES)))
        times = [pr.exec_time_ns for pr in prs if pr.exec_time_ns]
        exec_ns = max(times) if times else None
    except Exception as e:  # profile parse best-effort
        print("profile parse failed:", e)
    return results, exec_ns, neff_dir


def kernel(x, freqs_cis, wq, wk, wv, wo, q_norm_w, q_norm_b,
           k_norm_w, k_norm_b):
    global LAST_EXEC_NS, LAST_TRACE_DIR
    nc = _build()
    in_maps = _host_inputs(x, freqs_cis, wq, wk, wv, wo,
                           q_norm_w, q_norm_b, k_norm_w, k_norm_b)
    if PROFILE:
        results, LAST_EXEC_NS, LAST_TRACE_DIR = _run_profiled(nc, in_maps)
    else:
        res = bass_utils.run_bass_kernel_spmd(
            nc, in_maps, core_ids=list(range(NCORES)))
        results = res.results
        LAST_EXEC_NS = res.exec_time_ns
    acc = np.zeros((T, D), np.float32)
    for r in results:
        acc += np.asarray(r["out"], np.float32)
    return acc.reshape(B, S, D)
